# revision 22
# baseline (speedup 1.0000x reference)
"""Trainium2 Bass kernel for nn_ComplexHoloLinear.

Computes out = x @ Wr.T + cos(phase)[batch] * (x @ Wi.T) where Wr/Wi are
dense [4096, 4096] matrices assembled from COO duplicates (host-side
scatter-add, per the sharding hint's "replicate the assembled sparse
weight"), distributed by output-feature sharding: each of the 8 cores owns
512 output rows.

Device pipeline (per core), structured so the PE never starves:
  - cos(phase) on device (DVE range-fold + ACT Sin LUT; a dummy Sin on a
    memset tile preloads the ACT table off the critical path), then
    incremental combine deltas dlt[b] = cos_b - cos_{b-1}.
  - Weights live in SBUF as [128, 32*512] fp16 (WB = combined, WI = imag).
    All DMAs move chunk PAIRS (256-512 KiB) to halve dispatch (~600ns per
    dma_start of sequencer time) and the end-of-kernel per-DMA semaphore
    drain. During the first sweep the three input streams ride three
    rings: wr->sync, wi->gpsimd, x->scalar.
  - Batch 0 runs token-groups 0+1 jointly in one k-sweep (8 PSUM banks),
    so chunk consumption (~1.7us/chunk) stays below first-load arrival
    (~1.1us/chunk) and the PE never waits on assembly.
  - Per-batch combine WB += dlt_b * WI rides chunk-wise behind the
    previous batch's last sweep (WAR deps); no double buffer, no stall.
  - The last NS8=8 feature chunks run as fp8-e4m3 DoubleRow matmuls
    (2 fp8 MACs/cell/cycle): x pairs come pre-cast from the host, the
    combined weight pairs are cast fp16->fp8 on DVE each batch. With 8 of
    32 chunks in fp8 the output rel err is ~1.9e-2 (budget 2e-2) and the
    PE stream drops ~50us.
  - PSUM -> SBUF staging casts to fp16 on DVE into one [128, 2048] tile
    per token group -> single gpsimd out DMA; host upcasts to f32.
"""

import math
from contextlib import ExitStack

import numpy as np
import ml_dtypes

import concourse.bass as bass
import concourse.tile as tile
from concourse import bacc, mybir

F32 = mybir.dt.float32
F16 = mybir.dt.float16
F8E4 = mybir.dt.float8e4
ADD = mybir.AluOpType.add
MULT = mybir.AluOpType.mult


class Cfg:
    """Full-size problem config."""

    NCORES = 8
    NTOK = 8192       # B * S tokens
    NBATCH = 4        # batches (distinct cos factors)
    F = 4096          # in features (contraction)
    RTOT = 4096       # out features
    TOKG = 512        # tokens per matmul sweep group (psum tiles of 128)
    NS8 = 8           # trailing feature chunks computed in fp8 DoubleRow

    @property
    def RSH(self):    # rows per core
        return self.RTOT // self.NCORES

    @property
    def NK(self):     # feature chunks of 128
        return self.F // 128

    @property
    def NK16(self):   # fp16 chunks
        return self.NK - self.NS8

    @property
    def NP16(self):   # fp16 chunk pairs
        return self.NK16 // 2

    @property
    def ND8(self):    # fp8 chunk pairs (DoubleRow double-chunks)
        return self.NS8 // 2

    @property
    def NTG(self):    # token groups
        return self.NTOK // self.TOKG

    @property
    def WFREE(self):  # W tile free size
        return self.NK * self.RSH

    @property
    def DT_NP(self):
        return np.float16

    @property
    def DT(self):
        return F16


def build_body(ctx: ExitStack, tc: tile.TileContext, cfg: Cfg, aps: dict):
    nc = tc.nc
    xT2 = aps["xT2"]        # [NP16*NTG*128, 2*TOKG] fp16 chunk-pair tiles
    phase = aps["phase"]    # [1, NBATCH]
    wr, wi = aps["wr"], aps["wi"]  # [128, WFREE] fp16
    out = aps["out"]        # [NTOK, RSH] fp16
    xT8 = aps.get("xT8")    # [ND8*NTG*128, 2*TOKG] fp8 chunk-pair tiles

    RSH, NK, NB = cfg.RSH, cfg.NK, cfg.NBATCH
    TPG = cfg.TOKG // 128   # psum tiles per token group
    DT = cfg.DT
    NPAIR = NK // 2

    wpool = ctx.enter_context(tc.tile_pool(name="w", bufs=1))
    xpool = ctx.enter_context(tc.tile_pool(name="x", bufs=14))
    xqpool = ctx.enter_context(tc.tile_pool(name="xq", bufs=5))
    tpool = ctx.enter_context(tc.tile_pool(name="tmp", bufs=2))
    spool = ctx.enter_context(tc.tile_pool(name="stage", bufs=3))
    mpool = ctx.enter_context(tc.tile_pool(name="misc", bufs=1))
    pspool = ctx.enter_context(tc.tile_pool(name="ps", bufs=2, space="PSUM"))
    if cfg.ND8:
        x8pool = ctx.enter_context(tc.tile_pool(name="x8", bufs=6))
        x8qpool = ctx.enter_context(tc.tile_pool(name="x8q", bufs=3))

    # Dummy Sin on a memset tile: preloads the ACT Sin LUT while the phase
    # DMA is still in flight (the table load is ~1.3us and otherwise lands
    # on the cos critical path).
    dum = mpool.tile([128, 1], F32)
    nc.vector.memset(dum[:], 0.0)
    dums = mpool.tile([128, 1], F32)
    nc.scalar.activation(dums[:], dum[:], mybir.ActivationFunctionType.Sin)

    # --- cos(phase) = Sin(q), where q = fold(phase + pi/2) into [-pi, pi]
    # comes pre-folded and pre-broadcast from the host (pure relayout of 4
    # scalars; the transcendental stays on device).
    ph = mpool.tile([128, NB], F32)
    nc.sync.dma_start(out=ph[:], in_=phase[:, :])
    cos_t = mpool.tile([128, NB], F32)
    nc.scalar.activation(cos_t[:], ph[:], mybir.ActivationFunctionType.Sin)

    # incremental deltas for b>=1 (b=0 uses cos_t[:, 0:1] directly)
    dlt = mpool.tile([128, NB], F32)
    if NB > 1:
        nc.vector.tensor_tensor(out=dlt[:, 1:NB], in0=cos_t[:, 1:NB],
                                in1=cos_t[:, 0:NB - 1],
                                op=mybir.AluOpType.subtract)

    WB = wpool.tile([128, cfg.WFREE], DT)   # combined weight (starts as Wr)
    WI = wpool.tile([128, cfg.WFREE], DT)
    w8s = [wpool.tile([128, 2 * RSH], F8E4, name=f"w8_{kd}")
           for kd in range(cfg.ND8)]

    ntg_per_b = cfg.NTG // NB

    def mm_sweep(b, tgs):
        """One k-sweep over all chunk pairs for token groups `tgs` of
        batch b. tgs[0]==0 sweeps also run the per-batch weight combine;
        the b==0,tg==0 sweep additionally DMAs the weights in."""
        sweep0 = b == 0 and tgs[0] == 0
        combine = tgs[0] == 0
        gts = [b * ntg_per_b + tg for tg in tgs]
        pts = {}
        for i in range(len(tgs)):
            pts[i] = [pspool.tile([128, RSH], F32, space="PSUM",
                                  tag=f"ps{t}", name=f"ps{i}_{t}")
                      for t in range(TPG)]
        hw = [nc.scalar, nc.sync]

        def w_load_combine(k2):
            """Weight DMA (sweep0) + per-batch combine for chunk-pair k2.
            Pair 0 is graded into single chunks so the first matmul's
            weights arrive (and combine) as early as possible."""
            segs = ([slice(0, RSH), slice(RSH, 2 * RSH)] if k2 == 0 else
                    [slice(k2 * 2 * RSH, (k2 + 1) * 2 * RSH)])
            for si, sg in enumerate(segs):
                if sweep0:
                    # dedicated rings: wr=sync, wi=gpsimd, x=scalar — the
                    # joint sweep runs at ~85% of HBM bandwidth and mixing
                    # streams on one FIFO ring delays the weight pairs
                    nc.sync.dma_start(out=WB[:, sg], in_=wr[:, sg])
                    nc.gpsimd.dma_start(out=WI[:, sg], in_=wi[:, sg])
                if combine:
                    csc = cos_t[:, 0:1] if b == 0 else dlt[:, b:b + 1]
                    n = sg.stop - sg.start
                    tmp = tpool.tile([128, 2 * RSH], DT, name="tmp")
                    nc.vector.tensor_scalar(out=tmp[:, :n], in0=WI[:, sg],
                                            scalar1=csc, scalar2=None,
                                            op0=MULT)
                    nc.vector.tensor_tensor(out=WB[:, sg], in0=WB[:, sg],
                                            in1=tmp[:, :n], op=ADD)
            if combine and k2 >= cfg.NP16:
                pr = slice(k2 * 2 * RSH, (k2 + 1) * 2 * RSH)
                nc.vector.tensor_copy(w8s[k2 - cfg.NP16][:], WB[:, pr])

        def fp16_mms(i, xap, xoff, k2, first, last):
            for j in range(2):
                sl = slice((2 * k2 + j) * RSH, (2 * k2 + j + 1) * RSH)
                for t in range(TPG):
                    nc.tensor.matmul(
                        out=pts[i][t][:],
                        lhsT=xap[:, xoff + j * cfg.TOKG + t * 128:
                                 xoff + j * cfg.TOKG + (t + 1) * 128],
                        rhs=WB[:, sl],
                        start=(first and j == 0),
                        stop=(last and j == 1),
                    )

        def fp8_mms(i, xap, xoff, kd, last):
            w3 = w8s[kd][:].rearrange("p (j r) -> p j r", j=2)
            x3 = xap[:, xoff:xoff + 2 * cfg.TOKG].rearrange(
                "p (j w) -> p j w", j=2)
            for t in range(TPG):
                nc.tensor.matmul(
                    out=pts[i][t][:],
                    lhsT=x3[:, :, t * 128:(t + 1) * 128],
                    rhs=w3,
                    start=False, stop=last,
                    perf_mode=mybir.MatmulPerfMode.DoubleRow,
                )

        x4d = xT2.rearrange("(k g p) c -> k g p c", g=cfg.NTG, p=128)
        x8_4d = xT8.rearrange("(k g p) c -> k g p c", g=cfg.NTG, p=128) \
            if cfg.ND8 else None

        if len(gts) > 1 or sweep0:
            # pair-granular loads (three input streams on dedicated rings
            # while weights stream in). The fp8 pairs consume W twice as
            # fast as fp16 pairs, so interleave them 3:1 to keep the HBM
            # demand rate flat across the sweep (accumulation order is
            # free; pair 0 stays first / last pair stays last for the
            # PSUM start/stop flags).
            order = []
            f16l, f8l = list(range(cfg.NP16)), list(range(cfg.NP16, NPAIR))
            while f16l or f8l:
                order.extend(f16l[:3])
                del f16l[:3]
                if f8l:
                    order.append(f8l.pop(0))
            for k2 in order:
                fp8 = k2 >= cfg.NP16
                w_load_combine(k2)
                for i, gt in enumerate(gts):
                    xeng = nc.scalar if sweep0 else hw[(k2 + i) % 2]
                    if fp8:
                        kd = k2 - cfg.NP16
                        xt8 = x8pool.tile([128, 2 * cfg.TOKG], F8E4)
                        xeng.dma_start(out=xt8[:], in_=x8_4d[kd, gt, :, :])
                        fp8_mms(i, xt8[:], 0, kd, k2 == NPAIR - 1)
                    else:
                        xt = xpool.tile([128, 2 * cfg.TOKG], DT)
                        xeng.dma_start(out=xt[:], in_=x4d[k2, gt, :, :])
                        fp16_mms(i, xt[:], 0, k2, k2 == 0, False)
        else:
            # quad-granular x loads (halves DMA dispatch + end-of-kernel
            # semaphore drain); one 4D-AP DMA covers two chunk-pair blocks
            gt = gts[0]
            for q in range(NPAIR // 2):
                k2a = 2 * q
                fp8 = k2a >= cfg.NP16
                if combine:
                    w_load_combine(k2a)
                    w_load_combine(k2a + 1)
                if fp8:
                    kd = k2a - cfg.NP16
                    xt8 = x8qpool.tile([128, 4 * cfg.TOKG], F8E4)
                    hw[q % 2].dma_start(
                        out=xt8[:],
                        in_=x8_4d[kd:kd + 2, gt, :, :].rearrange(
                            "u p c -> p u c"))
                    fp8_mms(0, xt8[:], 0, kd, False)
                    fp8_mms(0, xt8[:], 2 * cfg.TOKG, kd + 1,
                            k2a + 1 == NPAIR - 1)
                else:
                    xt = xqpool.tile([128, 4 * cfg.TOKG], DT)
                    hw[q % 2].dma_start(
                        out=xt[:],
                        in_=x4d[k2a:k2a + 2, gt, :, :].rearrange(
                            "u p c -> p u c"))
                    fp16_mms(0, xt[:], 0, k2a, k2a == 0, False)
                    fp16_mms(0, xt[:], 2 * cfg.TOKG, k2a + 1, False, False)
        for i, gt in enumerate(gts):
            stg = spool.tile([128, TPG * RSH], DT)
            tok0 = gt * cfg.TOKG
            last_sweep = b == NB - 1 and gt == cfg.NTG - 1
            for t in range(TPG):
                if last_sweep:
                    # final group: split casts over DVE+ACT so they run in
                    # parallel, and send per-tile out DMAs as each lands —
                    # both shorten the end-of-kernel drain
                    if t % 2 == 0:
                        nc.vector.tensor_copy(stg[:, t * RSH:(t + 1) * RSH],
                                              pts[i][t][:])
                    else:
                        nc.scalar.copy(out=stg[:, t * RSH:(t + 1) * RSH],
                                       in_=pts[i][t][:])
                    tk = tok0 + t * 128
                    nc.gpsimd.dma_start(
                        out=out[tk:tk + 128, :],
                        in_=stg[:, t * RSH:(t + 1) * RSH])
                else:
                    nc.vector.tensor_copy(stg[:, t * RSH:(t + 1) * RSH],
                                          pts[i][t][:])
            if not last_sweep:
                dview = out[tok0:tok0 + cfg.TOKG, :].rearrange(
                    "(t p) r -> p t r", p=128)
                nc.gpsimd.dma_start(
                    out=dview, in_=stg[:].rearrange("p (t r) -> p t r",
                                                    t=TPG))

    for b in range(NB):
        sweeps = [[0, 1], [2], [3]] if b == 0 else [[0], [1], [2], [3]]
        for tgs in sweeps:
            mm_sweep(b, tgs)


def build_nc(cfg: Cfg):
    nc = bacc.Bacc("TRN2", target_bir_lowering=False, debug=False,
                   num_devices=cfg.NCORES)
    aps = {
        # x chunk-pair tiles: row block (k2*NTG + gt)*128 holds
        # [128 part, j*TOKG + t] = x[(2*k2+j)*128 + p, gt*TOKG + t]
        "xT2": nc.dram_tensor(
            "xT2", [cfg.NP16 * cfg.NTG * 128, 2 * cfg.TOKG], cfg.DT,
            kind="ExternalInput").ap(),
        "phase": nc.dram_tensor("phase", [128, cfg.NBATCH], F32,
                                kind="ExternalInput").ap(),
        "wr": nc.dram_tensor("wr", [128, cfg.WFREE], cfg.DT,
                             kind="ExternalInput").ap(),
        "wi": nc.dram_tensor("wi", [128, cfg.WFREE], cfg.DT,
                             kind="ExternalInput").ap(),
        "out": nc.dram_tensor("out", [cfg.NTOK, cfg.RSH], cfg.DT,
                              kind="ExternalOutput").ap(),
    }
    if cfg.ND8:
        # fp8 chunk-pair tiles for chunks NK16..NK-1, same block layout
        aps["xT8"] = nc.dram_tensor(
            "xT8", [cfg.ND8 * cfg.NTG * 128, 2 * cfg.TOKG], F8E4,
            kind="ExternalInput").ap()
    with tile.TileContext(nc) as tc:
        with ExitStack() as ctx:
            build_body(ctx, tc, cfg, aps)
    nc.compile()
    return nc


def host_prep(cfg: Cfg, x, rows, cols, w_real, w_imag, phase_angles):
    """Host prep: transpose/tile x (fp16 pairs + fp8 pairs for the
    DoubleRow chunks), scatter-add the COO edges into dense Wr/Wi, and
    slice/layout per-core W.T tiles. Returns per-core input maps."""
    x = np.ascontiguousarray(np.asarray(x, dtype=np.float32)).reshape(
        cfg.NTOK, cfg.F)
    xT = x.T  # [F, NTOK] f32

    def pair_tiles(xpart, np_dt):
        npair = xpart.shape[0] // 256
        return np.ascontiguousarray(
            xpart.reshape(npair, 2, 128, cfg.NTG, cfg.TOKG)
            .transpose(0, 3, 2, 1, 4)
        ).reshape(npair * cfg.NTG * 128, 2 * cfg.TOKG).astype(np_dt)

    xT2 = pair_tiles(xT[:cfg.NK16 * 128], cfg.DT_NP)

    rows = np.asarray(rows).astype(np.int64, copy=False)
    cols = np.asarray(cols).astype(np.int64, copy=False)
    lin = rows * cfg.F + cols
    ncell = cfg.RTOT * cfg.F
    Wr = np.bincount(lin, weights=np.asarray(w_real, np.float64),
                     minlength=ncell).astype(np.float32).reshape(
        cfg.RTOT, cfg.F)
    Wi = np.bincount(lin, weights=np.asarray(w_imag, np.float64),
                     minlength=ncell).astype(np.float32).reshape(
        cfg.RTOT, cfg.F)

    # per-core W.T layout: arr[c, p, k*RSH + r] = W[c*RSH + r, k*128 + p]
    def wt_layout(W):
        return np.ascontiguousarray(
            W.T.reshape(cfg.NK, 128, cfg.NCORES, cfg.RSH)
            .transpose(2, 1, 0, 3)
        ).reshape(cfg.NCORES, 128, cfg.WFREE).astype(cfg.DT_NP)

    wr_t = wt_layout(Wr)
    wi_t = wt_layout(Wi)

    # pre-folded Sin argument: q = phase + pi/2 wrapped into [-pi, pi],
    # broadcast to all 128 partitions (device computes cos = Sin(q))
    q = np.asarray(phase_angles, dtype=np.float32) + np.float32(math.pi / 2)
    q = np.where(q > math.pi, q - np.float32(2 * math.pi), q)
    phase_in = np.ascontiguousarray(
        np.broadcast_to(q.astype(np.float32), (128, cfg.NBATCH)))

    in_maps = []
    for cid in range(cfg.NCORES):
        m = {"xT2": xT2, "phase": phase_in,
             "wr": wr_t[cid], "wi": wi_t[cid]}
        in_maps.append(m)
    if cfg.ND8:
        xT8 = pair_tiles(xT[cfg.NK16 * 128:], ml_dtypes.float8_e4m3fn)
        for m in in_maps:
            m["xT8"] = xT8
    return in_maps


_NC_CACHE = {}
LAST_RESULTS = None  # BassKernelResults of the most recent kernel() call


def kernel(x, rows, cols, w_real, w_imag, phase_angles, out_features=4096,
           **_ignored):
    from concourse.bass_utils import run_bass_kernel_spmd

    global LAST_RESULTS
    cfg = Cfg()
    assert int(out_features) == cfg.RTOT

    if "nc" not in _NC_CACHE:
        _NC_CACHE["nc"] = build_nc(cfg)
    nc = _NC_CACHE["nc"]

    in_maps = host_prep(cfg, x, rows, cols, w_real, w_imag, phase_angles)
    res = run_bass_kernel_spmd(nc, in_maps, core_ids=list(range(cfg.NCORES)))
    LAST_RESULTS = res
    out = np.concatenate(
        [res.results[c]["out"].astype(np.float32)
         for c in range(cfg.NCORES)], axis=1)
    return out.reshape(cfg.NTOK // 2048, 2048, cfg.RTOT)


# revision 23
# speedup vs baseline: 1.0034x; 1.0034x over previous
"""Trainium2 Bass kernel for nn_ComplexHoloLinear.

Computes out = x @ Wr.T + cos(phase)[batch] * (x @ Wi.T) where Wr/Wi are
dense [4096, 4096] matrices assembled from COO duplicates (host-side
scatter-add, per the sharding hint's "replicate the assembled sparse
weight"), distributed by output-feature sharding: each of the 8 cores owns
512 output rows.

Device pipeline (per core), structured so the PE never starves:
  - cos(phase) on device (DVE range-fold + ACT Sin LUT; a dummy Sin on a
    memset tile preloads the ACT table off the critical path), then
    incremental combine deltas dlt[b] = cos_b - cos_{b-1}.
  - Weights live in SBUF as [128, 32*512] fp16 (WB = combined, WI = imag).
    All DMAs move chunk PAIRS (256-512 KiB) to halve dispatch (~600ns per
    dma_start of sequencer time) and the end-of-kernel per-DMA semaphore
    drain. During the first sweep the three input streams ride three
    rings: wr->sync, wi->gpsimd, x->scalar.
  - Batch 0 runs token-groups 0+1 jointly in one k-sweep (8 PSUM banks),
    so chunk consumption (~1.7us/chunk) stays below first-load arrival
    (~1.1us/chunk) and the PE never waits on assembly.
  - Per-batch combine WB += dlt_b * WI rides chunk-wise behind the
    previous batch's last sweep (WAR deps); no double buffer, no stall.
  - The last NS8=8 feature chunks run as fp8-e4m3 DoubleRow matmuls
    (2 fp8 MACs/cell/cycle): x pairs come pre-cast from the host, the
    combined weight pairs are cast fp16->fp8 on DVE each batch. With 8 of
    32 chunks in fp8 the output rel err is ~1.9e-2 (budget 2e-2) and the
    PE stream drops ~50us.
  - PSUM -> SBUF staging casts to fp16 on DVE into one [128, 2048] tile
    per token group -> single gpsimd out DMA; host upcasts to f32.
"""

import math
from contextlib import ExitStack

import numpy as np
import ml_dtypes

import concourse.bass as bass
import concourse.tile as tile
from concourse import bacc, mybir

F32 = mybir.dt.float32
F16 = mybir.dt.float16
F8E4 = mybir.dt.float8e4
ADD = mybir.AluOpType.add
MULT = mybir.AluOpType.mult


class Cfg:
    """Full-size problem config."""

    NCORES = 8
    NTOK = 8192       # B * S tokens
    NBATCH = 4        # batches (distinct cos factors)
    F = 4096          # in features (contraction)
    RTOT = 4096       # out features
    TOKG = 512        # tokens per matmul sweep group (psum tiles of 128)
    NS8 = 8           # trailing feature chunks computed in fp8 DoubleRow

    @property
    def RSH(self):    # rows per core
        return self.RTOT // self.NCORES

    @property
    def NK(self):     # feature chunks of 128
        return self.F // 128

    @property
    def NK16(self):   # fp16 chunks
        return self.NK - self.NS8

    @property
    def NP16(self):   # fp16 chunk pairs
        return self.NK16 // 2

    @property
    def ND8(self):    # fp8 chunk pairs (DoubleRow double-chunks)
        return self.NS8 // 2

    @property
    def NTG(self):    # token groups
        return self.NTOK // self.TOKG

    @property
    def WFREE(self):  # W tile free size
        return self.NK * self.RSH

    @property
    def DT_NP(self):
        return np.float16

    @property
    def DT(self):
        return F16


def build_body(ctx: ExitStack, tc: tile.TileContext, cfg: Cfg, aps: dict):
    nc = tc.nc
    xT2 = aps["xT2"]        # [NP16*NTG*128, 2*TOKG] fp16 chunk-pair tiles
    phase = aps["phase"]    # [1, NBATCH]
    wr, wi = aps["wr"], aps["wi"]  # [128, WFREE] fp16
    out = aps["out"]        # [NTOK, RSH] fp16
    xT8 = aps.get("xT8")    # [ND8*NTG*128, 2*TOKG] fp8 chunk-pair tiles

    RSH, NK, NB = cfg.RSH, cfg.NK, cfg.NBATCH
    TPG = cfg.TOKG // 128   # psum tiles per token group
    DT = cfg.DT
    NPAIR = NK // 2

    wpool = ctx.enter_context(tc.tile_pool(name="w", bufs=1))
    xpool = ctx.enter_context(tc.tile_pool(name="x", bufs=14))
    xqpool = ctx.enter_context(tc.tile_pool(name="xq", bufs=5))
    tpool = ctx.enter_context(tc.tile_pool(name="tmp", bufs=2))
    spool = ctx.enter_context(tc.tile_pool(name="stage", bufs=3))
    mpool = ctx.enter_context(tc.tile_pool(name="misc", bufs=1))
    pspool = ctx.enter_context(tc.tile_pool(name="ps", bufs=2, space="PSUM"))
    if cfg.ND8:
        x8pool = ctx.enter_context(tc.tile_pool(name="x8", bufs=6))
        x8qpool = ctx.enter_context(tc.tile_pool(name="x8q", bufs=3))

    # Dummy Sin on a memset tile: preloads the ACT Sin LUT while the phase
    # DMA is still in flight (the table load is ~1.3us and otherwise lands
    # on the cos critical path).
    dum = mpool.tile([128, 1], F32)
    nc.vector.memset(dum[:], 0.0)
    dums = mpool.tile([128, 1], F32)
    nc.scalar.activation(dums[:], dum[:], mybir.ActivationFunctionType.Sin)

    # --- cos(phase) = Sin(q), where q = fold(phase + pi/2) into [-pi, pi]
    # comes pre-folded and pre-broadcast from the host (pure relayout of 4
    # scalars; the transcendental stays on device).
    ph = mpool.tile([128, NB], F32)
    nc.sync.dma_start(out=ph[:], in_=phase[:, :])
    cos_t = mpool.tile([128, NB], F32)
    nc.scalar.activation(cos_t[:], ph[:], mybir.ActivationFunctionType.Sin)

    # incremental deltas for b>=1 (b=0 uses cos_t[:, 0:1] directly)
    dlt = mpool.tile([128, NB], F32)
    if NB > 1:
        nc.vector.tensor_tensor(out=dlt[:, 1:NB], in0=cos_t[:, 1:NB],
                                in1=cos_t[:, 0:NB - 1],
                                op=mybir.AluOpType.subtract)

    WB = wpool.tile([128, cfg.WFREE], DT)   # combined weight (starts as Wr)
    WI = wpool.tile([128, cfg.WFREE], DT)
    w8s = [wpool.tile([128, 2 * RSH], F8E4, name=f"w8_{kd}")
           for kd in range(cfg.ND8)]

    ntg_per_b = cfg.NTG // NB

    def mm_sweep(b, tgs):
        """One k-sweep over all chunk pairs for token groups `tgs` of
        batch b. tgs[0]==0 sweeps also run the per-batch weight combine;
        the b==0,tg==0 sweep additionally DMAs the weights in."""
        sweep0 = b == 0 and tgs[0] == 0
        combine = tgs[0] == 0
        gts = [b * ntg_per_b + tg for tg in tgs]
        pts = {}
        for i in range(len(tgs)):
            pts[i] = [pspool.tile([128, RSH], F32, space="PSUM",
                                  tag=f"ps{t}", name=f"ps{i}_{t}")
                      for t in range(TPG)]
        hw = [nc.scalar, nc.sync]

        def w_load_combine(k2):
            """Weight DMA (sweep0) + per-batch combine for chunk-pair k2.
            Pair 0 is graded into single chunks so the first matmul's
            weights arrive (and combine) as early as possible."""
            segs = ([slice(0, RSH), slice(RSH, 2 * RSH)] if k2 == 0 else
                    [slice(k2 * 2 * RSH, (k2 + 1) * 2 * RSH)])
            for si, sg in enumerate(segs):
                if sweep0:
                    # dedicated rings: wr=sync, wi=gpsimd, x=scalar — the
                    # joint sweep runs at ~85% of HBM bandwidth and mixing
                    # streams on one FIFO ring delays the weight pairs
                    nc.sync.dma_start(out=WB[:, sg], in_=wr[:, sg])
                    nc.gpsimd.dma_start(out=WI[:, sg], in_=wi[:, sg])
                if combine:
                    csc = cos_t[:, 0:1] if b == 0 else dlt[:, b:b + 1]
                    n = sg.stop - sg.start
                    tmp = tpool.tile([128, 2 * RSH], DT, name="tmp")
                    nc.vector.tensor_scalar(out=tmp[:, :n], in0=WI[:, sg],
                                            scalar1=csc, scalar2=None,
                                            op0=MULT)
                    nc.vector.tensor_tensor(out=WB[:, sg], in0=WB[:, sg],
                                            in1=tmp[:, :n], op=ADD)
            if combine and k2 >= cfg.NP16:
                pr = slice(k2 * 2 * RSH, (k2 + 1) * 2 * RSH)
                nc.vector.tensor_copy(w8s[k2 - cfg.NP16][:], WB[:, pr])

        def fp16_mms(i, xap, xoff, k2, first, last):
            for j in range(2):
                sl = slice((2 * k2 + j) * RSH, (2 * k2 + j + 1) * RSH)
                for t in range(TPG):
                    nc.tensor.matmul(
                        out=pts[i][t][:],
                        lhsT=xap[:, xoff + j * cfg.TOKG + t * 128:
                                 xoff + j * cfg.TOKG + (t + 1) * 128],
                        rhs=WB[:, sl],
                        start=(first and j == 0),
                        stop=(last and j == 1),
                    )

        def fp8_mms(i, xap, xoff, kd, last):
            w3 = w8s[kd][:].rearrange("p (j r) -> p j r", j=2)
            x3 = xap[:, xoff:xoff + 2 * cfg.TOKG].rearrange(
                "p (j w) -> p j w", j=2)
            for t in range(TPG):
                nc.tensor.matmul(
                    out=pts[i][t][:],
                    lhsT=x3[:, :, t * 128:(t + 1) * 128],
                    rhs=w3,
                    start=False, stop=last,
                    perf_mode=mybir.MatmulPerfMode.DoubleRow,
                )

        x4d = xT2.rearrange("(k g p) c -> k g p c", g=cfg.NTG, p=128)
        x8_4d = xT8.rearrange("(k g p) c -> k g p c", g=cfg.NTG, p=128) \
            if cfg.ND8 else None

        if len(gts) > 1 or sweep0:
            # pair-granular loads (three input streams on dedicated rings
            # while weights stream in)
            for k2 in range(NPAIR):
                fp8 = k2 >= cfg.NP16
                w_load_combine(k2)
                for i, gt in enumerate(gts):
                    xeng = nc.scalar if sweep0 else hw[(k2 + i) % 2]
                    if fp8:
                        kd = k2 - cfg.NP16
                        xt8 = x8pool.tile([128, 2 * cfg.TOKG], F8E4)
                        xeng.dma_start(out=xt8[:], in_=x8_4d[kd, gt, :, :])
                        fp8_mms(i, xt8[:], 0, kd, k2 == NPAIR - 1)
                    else:
                        xt = xpool.tile([128, 2 * cfg.TOKG], DT)
                        xeng.dma_start(out=xt[:], in_=x4d[k2, gt, :, :])
                        fp16_mms(i, xt[:], 0, k2, k2 == 0, False)
        else:
            # quad-granular x loads (halves DMA dispatch + end-of-kernel
            # semaphore drain); one 4D-AP DMA covers two chunk-pair blocks
            gt = gts[0]
            for q in range(NPAIR // 2):
                k2a = 2 * q
                fp8 = k2a >= cfg.NP16
                if combine:
                    w_load_combine(k2a)
                    w_load_combine(k2a + 1)
                if fp8:
                    kd = k2a - cfg.NP16
                    xt8 = x8qpool.tile([128, 4 * cfg.TOKG], F8E4)
                    hw[q % 2].dma_start(
                        out=xt8[:],
                        in_=x8_4d[kd:kd + 2, gt, :, :].rearrange(
                            "u p c -> p u c"))
                    fp8_mms(0, xt8[:], 0, kd, False)
                    fp8_mms(0, xt8[:], 2 * cfg.TOKG, kd + 1,
                            k2a + 1 == NPAIR - 1)
                else:
                    xt = xqpool.tile([128, 4 * cfg.TOKG], DT)
                    hw[q % 2].dma_start(
                        out=xt[:],
                        in_=x4d[k2a:k2a + 2, gt, :, :].rearrange(
                            "u p c -> p u c"))
                    fp16_mms(0, xt[:], 0, k2a, k2a == 0, False)
                    fp16_mms(0, xt[:], 2 * cfg.TOKG, k2a + 1, False, False)
        for i, gt in enumerate(gts):
            stg = spool.tile([128, TPG * RSH], DT)
            tok0 = gt * cfg.TOKG
            last_sweep = b == NB - 1 and gt == cfg.NTG - 1
            for t in range(TPG):
                if last_sweep:
                    # final group: split casts over DVE+ACT so they run in
                    # parallel, and send per-tile out DMAs as each lands —
                    # both shorten the end-of-kernel drain
                    if t % 2 == 0:
                        nc.vector.tensor_copy(stg[:, t * RSH:(t + 1) * RSH],
                                              pts[i][t][:])
                    else:
                        nc.scalar.copy(out=stg[:, t * RSH:(t + 1) * RSH],
                                       in_=pts[i][t][:])
                    tk = tok0 + t * 128
                    nc.gpsimd.dma_start(
                        out=out[tk:tk + 128, :],
                        in_=stg[:, t * RSH:(t + 1) * RSH])
                else:
                    nc.vector.tensor_copy(stg[:, t * RSH:(t + 1) * RSH],
                                          pts[i][t][:])
            if not last_sweep:
                dview = out[tok0:tok0 + cfg.TOKG, :].rearrange(
                    "(t p) r -> p t r", p=128)
                nc.gpsimd.dma_start(
                    out=dview, in_=stg[:].rearrange("p (t r) -> p t r",
                                                    t=TPG))

    for b in range(NB):
        sweeps = [[0, 1], [2], [3]] if b == 0 else [[0], [1], [2], [3]]
        for tgs in sweeps:
            mm_sweep(b, tgs)


def build_nc(cfg: Cfg):
    nc = bacc.Bacc("TRN2", target_bir_lowering=False, debug=False,
                   num_devices=cfg.NCORES)
    aps = {
        # x chunk-pair tiles: row block (k2*NTG + gt)*128 holds
        # [128 part, j*TOKG + t] = x[(2*k2+j)*128 + p, gt*TOKG + t]
        "xT2": nc.dram_tensor(
            "xT2", [cfg.NP16 * cfg.NTG * 128, 2 * cfg.TOKG], cfg.DT,
            kind="ExternalInput").ap(),
        "phase": nc.dram_tensor("phase", [128, cfg.NBATCH], F32,
                                kind="ExternalInput").ap(),
        "wr": nc.dram_tensor("wr", [128, cfg.WFREE], cfg.DT,
                             kind="ExternalInput").ap(),
        "wi": nc.dram_tensor("wi", [128, cfg.WFREE], cfg.DT,
                             kind="ExternalInput").ap(),
        "out": nc.dram_tensor("out", [cfg.NTOK, cfg.RSH], cfg.DT,
                              kind="ExternalOutput").ap(),
    }
    if cfg.ND8:
        # fp8 chunk-pair tiles for chunks NK16..NK-1, same block layout
        aps["xT8"] = nc.dram_tensor(
            "xT8", [cfg.ND8 * cfg.NTG * 128, 2 * cfg.TOKG], F8E4,
            kind="ExternalInput").ap()
    with tile.TileContext(nc) as tc:
        with ExitStack() as ctx:
            build_body(ctx, tc, cfg, aps)
    nc.compile()
    return nc


def host_prep(cfg: Cfg, x, rows, cols, w_real, w_imag, phase_angles):
    """Host prep: transpose/tile x (fp16 pairs + fp8 pairs for the
    DoubleRow chunks), scatter-add the COO edges into dense Wr/Wi, and
    slice/layout per-core W.T tiles. Returns per-core input maps."""
    x = np.ascontiguousarray(np.asarray(x, dtype=np.float32)).reshape(
        cfg.NTOK, cfg.F)
    xT = x.T  # [F, NTOK] f32

    def pair_tiles(xpart, np_dt):
        npair = xpart.shape[0] // 256
        return np.ascontiguousarray(
            xpart.reshape(npair, 2, 128, cfg.NTG, cfg.TOKG)
            .transpose(0, 3, 2, 1, 4)
        ).reshape(npair * cfg.NTG * 128, 2 * cfg.TOKG).astype(np_dt)

    xT2 = pair_tiles(xT[:cfg.NK16 * 128], cfg.DT_NP)

    rows = np.asarray(rows).astype(np.int64, copy=False)
    cols = np.asarray(cols).astype(np.int64, copy=False)
    lin = rows * cfg.F + cols
    ncell = cfg.RTOT * cfg.F
    Wr = np.bincount(lin, weights=np.asarray(w_real, np.float64),
                     minlength=ncell).astype(np.float32).reshape(
        cfg.RTOT, cfg.F)
    Wi = np.bincount(lin, weights=np.asarray(w_imag, np.float64),
                     minlength=ncell).astype(np.float32).reshape(
        cfg.RTOT, cfg.F)

    # per-core W.T layout: arr[c, p, k*RSH + r] = W[c*RSH + r, k*128 + p]
    def wt_layout(W):
        return np.ascontiguousarray(
            W.T.reshape(cfg.NK, 128, cfg.NCORES, cfg.RSH)
            .transpose(2, 1, 0, 3)
        ).reshape(cfg.NCORES, 128, cfg.WFREE).astype(cfg.DT_NP)

    wr_t = wt_layout(Wr)
    wi_t = wt_layout(Wi)

    # pre-folded Sin argument: q = phase + pi/2 wrapped into [-pi, pi],
    # broadcast to all 128 partitions (device computes cos = Sin(q))
    q = np.asarray(phase_angles, dtype=np.float32) + np.float32(math.pi / 2)
    q = np.where(q > math.pi, q - np.float32(2 * math.pi), q)
    phase_in = np.ascontiguousarray(
        np.broadcast_to(q.astype(np.float32), (128, cfg.NBATCH)))

    in_maps = []
    for cid in range(cfg.NCORES):
        m = {"xT2": xT2, "phase": phase_in,
             "wr": wr_t[cid], "wi": wi_t[cid]}
        in_maps.append(m)
    if cfg.ND8:
        xT8 = pair_tiles(xT[cfg.NK16 * 128:], ml_dtypes.float8_e4m3fn)
        for m in in_maps:
            m["xT8"] = xT8
    return in_maps


_NC_CACHE = {}
LAST_RESULTS = None  # BassKernelResults of the most recent kernel() call


def kernel(x, rows, cols, w_real, w_imag, phase_angles, out_features=4096,
           **_ignored):
    from concourse.bass_utils import run_bass_kernel_spmd

    global LAST_RESULTS
    cfg = Cfg()
    assert int(out_features) == cfg.RTOT

    if "nc" not in _NC_CACHE:
        _NC_CACHE["nc"] = build_nc(cfg)
    nc = _NC_CACHE["nc"]

    in_maps = host_prep(cfg, x, rows, cols, w_real, w_imag, phase_angles)
    res = run_bass_kernel_spmd(nc, in_maps, core_ids=list(range(cfg.NCORES)))
    LAST_RESULTS = res
    out = np.concatenate(
        [res.results[c]["out"].astype(np.float32)
         for c in range(cfg.NCORES)], axis=1)
    return out.reshape(cfg.NTOK // 2048, 2048, cfg.RTOT)


# revision 26
# speedup vs baseline: 1.0115x; 1.0081x over previous
"""Trainium2 Bass kernel for nn_ComplexHoloLinear.

Computes out = x @ Wr.T + cos(phase)[batch] * (x @ Wi.T) where Wr/Wi are
dense [4096, 4096] matrices assembled from COO duplicates (host-side
scatter-add, per the sharding hint's "replicate the assembled sparse
weight"), distributed by output-feature sharding: each of the 8 cores owns
512 output rows.

Device pipeline (per core), structured so the PE never starves:
  - cos(phase) on device (DVE range-fold + ACT Sin LUT; a dummy Sin on a
    memset tile preloads the ACT table off the critical path), then
    incremental combine deltas dlt[b] = cos_b - cos_{b-1}.
  - Weights live in SBUF as [128, 32*512] fp16 (WB = combined, WI = imag).
    All DMAs move chunk PAIRS (256-512 KiB) to halve dispatch (~600ns per
    dma_start of sequencer time) and the end-of-kernel per-DMA semaphore
    drain. During the first sweep the three input streams ride three
    rings: wr->sync, wi->gpsimd, x->scalar.
  - Batch 0 runs token-groups 0+1 jointly in one k-sweep (8 PSUM banks),
    so chunk consumption (~1.7us/chunk) stays below first-load arrival
    (~1.1us/chunk) and the PE never waits on assembly.
  - Per-batch combine WB += dlt_b * WI rides chunk-wise behind the
    previous batch's last sweep (WAR deps); no double buffer, no stall.
  - The last NS8=8 feature chunks run as fp8-e4m3 DoubleRow matmuls
    (2 fp8 MACs/cell/cycle): x pairs come pre-cast from the host, the
    combined weight pairs are cast fp16->fp8 on DVE each batch. With 8 of
    32 chunks in fp8 the output rel err is ~1.9e-2 (budget 2e-2) and the
    PE stream drops ~50us.
  - PSUM -> SBUF staging casts to fp16 on DVE into one [128, 2048] tile
    per token group -> single gpsimd out DMA; host upcasts to f32.
"""

import math
from contextlib import ExitStack

import numpy as np
import ml_dtypes

import concourse.bass as bass
import concourse.tile as tile
from concourse import bacc, mybir

F32 = mybir.dt.float32
F16 = mybir.dt.float16
F8E4 = mybir.dt.float8e4
ADD = mybir.AluOpType.add
MULT = mybir.AluOpType.mult


class Cfg:
    """Full-size problem config."""

    NCORES = 8
    NTOK = 8192       # B * S tokens
    NBATCH = 4        # batches (distinct cos factors)
    F = 4096          # in features (contraction)
    RTOT = 4096       # out features
    TOKG = 512        # tokens per matmul sweep group (psum tiles of 128)
    NS8 = 8           # trailing feature chunks computed in fp8 DoubleRow

    @property
    def RSH(self):    # rows per core
        return self.RTOT // self.NCORES

    @property
    def NK(self):     # feature chunks of 128
        return self.F // 128

    @property
    def NK16(self):   # fp16 chunks
        return self.NK - self.NS8

    @property
    def NP16(self):   # fp16 chunk pairs
        return self.NK16 // 2

    @property
    def ND8(self):    # fp8 chunk pairs (DoubleRow double-chunks)
        return self.NS8 // 2

    @property
    def NTG(self):    # token groups
        return self.NTOK // self.TOKG

    @property
    def WFREE(self):  # W tile free size
        return self.NK * self.RSH

    @property
    def DT_NP(self):
        return np.float16

    @property
    def DT(self):
        return F16


def build_body(ctx: ExitStack, tc: tile.TileContext, cfg: Cfg, aps: dict):
    nc = tc.nc
    xT2 = aps["xT2"]        # [NP16*NTG*128, 2*TOKG] fp16 chunk-pair tiles
    phase = aps["phase"]    # [1, NBATCH]
    wr, wi = aps["wr"], aps["wi"]  # [128, WFREE] fp16
    out = aps["out"]        # [NTOK, RSH] fp16
    xT8 = aps.get("xT8")    # [ND8*NTG*128, 2*TOKG] fp8 chunk-pair tiles

    RSH, NK, NB = cfg.RSH, cfg.NK, cfg.NBATCH
    TPG = cfg.TOKG // 128   # psum tiles per token group
    DT = cfg.DT
    NPAIR = NK // 2

    wpool = ctx.enter_context(tc.tile_pool(name="w", bufs=1))
    xpool = ctx.enter_context(tc.tile_pool(name="x", bufs=16))
    xqpool = ctx.enter_context(tc.tile_pool(name="xq", bufs=5))
    tpool = ctx.enter_context(tc.tile_pool(name="tmp", bufs=2))
    spool = ctx.enter_context(tc.tile_pool(name="stage", bufs=3))
    mpool = ctx.enter_context(tc.tile_pool(name="misc", bufs=1))
    pspool = ctx.enter_context(tc.tile_pool(name="ps", bufs=2, space="PSUM"))
    if cfg.ND8:
        x8pool = ctx.enter_context(tc.tile_pool(name="x8", bufs=8))
        x8qpool = ctx.enter_context(tc.tile_pool(name="x8q", bufs=3))

    # Dummy Sin on a memset tile: preloads the ACT Sin LUT while the phase
    # DMA is still in flight (the table load is ~1.3us and otherwise lands
    # on the cos critical path).
    dum = mpool.tile([128, 1], F32)
    nc.vector.memset(dum[:], 0.0)
    dums = mpool.tile([128, 1], F32)
    nc.scalar.activation(dums[:], dum[:], mybir.ActivationFunctionType.Sin)

    # --- cos(phase) = Sin(q), where q = fold(phase + pi/2) into [-pi, pi]
    # comes pre-folded and pre-broadcast from the host (pure relayout of 4
    # scalars; the transcendental stays on device).
    ph = mpool.tile([128, NB], F32)
    nc.sync.dma_start(out=ph[:], in_=phase[:, :])
    cos_t = mpool.tile([128, NB], F32)
    nc.scalar.activation(cos_t[:], ph[:], mybir.ActivationFunctionType.Sin)

    # incremental deltas for b>=1 (b=0 uses cos_t[:, 0:1] directly)
    dlt = mpool.tile([128, NB], F32)
    if NB > 1:
        nc.vector.tensor_tensor(out=dlt[:, 1:NB], in0=cos_t[:, 1:NB],
                                in1=cos_t[:, 0:NB - 1],
                                op=mybir.AluOpType.subtract)

    WB = wpool.tile([128, cfg.WFREE], DT)   # combined weight (starts as Wr)
    WI = wpool.tile([128, cfg.WFREE], DT)
    w8s = [wpool.tile([128, 2 * RSH], F8E4, name=f"w8_{kd}")
           for kd in range(cfg.ND8)]

    ntg_per_b = cfg.NTG // NB

    def mm_sweep(b, tgs):
        """One k-sweep over all chunk pairs for token groups `tgs` of
        batch b. tgs[0]==0 sweeps also run the per-batch weight combine;
        the b==0,tg==0 sweep additionally DMAs the weights in."""
        sweep0 = b == 0 and tgs[0] == 0
        combine = tgs[0] == 0
        gts = [b * ntg_per_b + tg for tg in tgs]
        pts = {}
        for i in range(len(tgs)):
            pts[i] = [pspool.tile([128, RSH], F32, space="PSUM",
                                  tag=f"ps{t}", name=f"ps{i}_{t}")
                      for t in range(TPG)]
        hw = [nc.scalar, nc.sync]

        def w_load_combine(k2):
            """Weight DMA (sweep0) + per-batch combine for chunk-pair k2.
            Pair 0 is graded into single chunks so the first matmul's
            weights arrive (and combine) as early as possible."""
            segs = ([slice(0, RSH), slice(RSH, 2 * RSH)] if k2 == 0 else
                    [slice(k2 * 2 * RSH, (k2 + 1) * 2 * RSH)])
            for si, sg in enumerate(segs):
                if sweep0:
                    # dedicated rings: wr=sync, wi=gpsimd, x=scalar — the
                    # joint sweep runs at ~85% of HBM bandwidth and mixing
                    # streams on one FIFO ring delays the weight pairs
                    nc.sync.dma_start(out=WB[:, sg], in_=wr[:, sg])
                    nc.gpsimd.dma_start(out=WI[:, sg], in_=wi[:, sg])
                if combine:
                    csc = cos_t[:, 0:1] if b == 0 else dlt[:, b:b + 1]
                    n = sg.stop - sg.start
                    tmp = tpool.tile([128, 2 * RSH], DT, name="tmp")
                    nc.vector.tensor_scalar(out=tmp[:, :n], in0=WI[:, sg],
                                            scalar1=csc, scalar2=None,
                                            op0=MULT)
                    nc.vector.tensor_tensor(out=WB[:, sg], in0=WB[:, sg],
                                            in1=tmp[:, :n], op=ADD)
            if combine and k2 >= cfg.NP16:
                pr = slice(k2 * 2 * RSH, (k2 + 1) * 2 * RSH)
                nc.vector.tensor_copy(w8s[k2 - cfg.NP16][:], WB[:, pr])

        def fp16_mms(i, xap, xoff, k2, first, last):
            for j in range(2):
                sl = slice((2 * k2 + j) * RSH, (2 * k2 + j + 1) * RSH)
                for t in range(TPG):
                    nc.tensor.matmul(
                        out=pts[i][t][:],
                        lhsT=xap[:, xoff + j * cfg.TOKG + t * 128:
                                 xoff + j * cfg.TOKG + (t + 1) * 128],
                        rhs=WB[:, sl],
                        start=(first and j == 0),
                        stop=(last and j == 1),
                    )

        def fp8_mms(i, xap, xoff, kd, last):
            w3 = w8s[kd][:].rearrange("p (j r) -> p j r", j=2)
            x3 = xap[:, xoff:xoff + 2 * cfg.TOKG].rearrange(
                "p (j w) -> p j w", j=2)
            for t in range(TPG):
                nc.tensor.matmul(
                    out=pts[i][t][:],
                    lhsT=x3[:, :, t * 128:(t + 1) * 128],
                    rhs=w3,
                    start=False, stop=last,
                    perf_mode=mybir.MatmulPerfMode.DoubleRow,
                )

        x4d = xT2.rearrange("(k g p) c -> k g p c", g=cfg.NTG, p=128)
        x8_4d = xT8.rearrange("(k g p) c -> k g p c", g=cfg.NTG, p=128) \
            if cfg.ND8 else None

        if len(gts) > 1 or sweep0:
            # pair-granular loads (three input streams on dedicated rings
            # while weights stream in)
            for k2 in range(NPAIR):
                fp8 = k2 >= cfg.NP16
                w_load_combine(k2)
                for i, gt in enumerate(gts):
                    xeng = nc.scalar if sweep0 else hw[(k2 + i) % 2]
                    if fp8:
                        kd = k2 - cfg.NP16
                        xt8 = x8pool.tile([128, 2 * cfg.TOKG], F8E4)
                        xeng.dma_start(out=xt8[:], in_=x8_4d[kd, gt, :, :])
                        fp8_mms(i, xt8[:], 0, kd, k2 == NPAIR - 1)
                    else:
                        xt = xpool.tile([128, 2 * cfg.TOKG], DT)
                        xeng.dma_start(out=xt[:], in_=x4d[k2, gt, :, :])
                        fp16_mms(i, xt[:], 0, k2, k2 == 0, False)
        else:
            # quad-granular x loads (halves DMA dispatch + end-of-kernel
            # semaphore drain); one 4D-AP DMA covers two chunk-pair blocks
            gt = gts[0]
            for q in range(NPAIR // 2):
                k2a = 2 * q
                fp8 = k2a >= cfg.NP16
                if combine:
                    w_load_combine(k2a)
                    w_load_combine(k2a + 1)
                if fp8:
                    kd = k2a - cfg.NP16
                    xt8 = x8qpool.tile([128, 4 * cfg.TOKG], F8E4)
                    hw[q % 2].dma_start(
                        out=xt8[:],
                        in_=x8_4d[kd:kd + 2, gt, :, :].rearrange(
                            "u p c -> p u c"))
                    fp8_mms(0, xt8[:], 0, kd, False)
                    fp8_mms(0, xt8[:], 2 * cfg.TOKG, kd + 1,
                            k2a + 1 == NPAIR - 1)
                else:
                    xt = xqpool.tile([128, 4 * cfg.TOKG], DT)
                    hw[q % 2].dma_start(
                        out=xt[:],
                        in_=x4d[k2a:k2a + 2, gt, :, :].rearrange(
                            "u p c -> p u c"))
                    fp16_mms(0, xt[:], 0, k2a, k2a == 0, False)
                    fp16_mms(0, xt[:], 2 * cfg.TOKG, k2a + 1, False, False)
        for i, gt in enumerate(gts):
            stg = spool.tile([128, TPG * RSH], DT)
            tok0 = gt * cfg.TOKG
            last_sweep = b == NB - 1 and gt == cfg.NTG - 1
            for t in range(TPG):
                if last_sweep:
                    # final group: split casts over DVE+ACT so they run in
                    # parallel, and send per-tile out DMAs as each lands —
                    # both shorten the end-of-kernel drain
                    if t % 2 == 0:
                        nc.vector.tensor_copy(stg[:, t * RSH:(t + 1) * RSH],
                                              pts[i][t][:])
                    else:
                        nc.scalar.copy(out=stg[:, t * RSH:(t + 1) * RSH],
                                       in_=pts[i][t][:])
                    tk = tok0 + t * 128
                    hw[t % 2].dma_start(
                        out=out[tk:tk + 128, :],
                        in_=stg[:, t * RSH:(t + 1) * RSH])
                else:
                    nc.vector.tensor_copy(stg[:, t * RSH:(t + 1) * RSH],
                                          pts[i][t][:])
            if not last_sweep:
                # outs ride the HWDGE rings (idle mid-stream; faster
                # completion receipt than SWDGE and keeps the gpsimd
                # end-of-kernel semaphore ladder off the critical tail)
                dview = out[tok0:tok0 + cfg.TOKG, :].rearrange(
                    "(t p) r -> p t r", p=128)
                hw[gt % 2].dma_start(
                    out=dview, in_=stg[:].rearrange("p (t r) -> p t r",
                                                    t=TPG))

    for b in range(NB):
        sweeps = [[0, 1], [2], [3]] if b == 0 else [[0], [1], [2], [3]]
        for tgs in sweeps:
            mm_sweep(b, tgs)


def build_nc(cfg: Cfg):
    nc = bacc.Bacc("TRN2", target_bir_lowering=False, debug=False,
                   num_devices=cfg.NCORES)
    aps = {
        # x chunk-pair tiles: row block (k2*NTG + gt)*128 holds
        # [128 part, j*TOKG + t] = x[(2*k2+j)*128 + p, gt*TOKG + t]
        "xT2": nc.dram_tensor(
            "xT2", [cfg.NP16 * cfg.NTG * 128, 2 * cfg.TOKG], cfg.DT,
            kind="ExternalInput").ap(),
        "phase": nc.dram_tensor("phase", [128, cfg.NBATCH], F32,
                                kind="ExternalInput").ap(),
        "wr": nc.dram_tensor("wr", [128, cfg.WFREE], cfg.DT,
                             kind="ExternalInput").ap(),
        "wi": nc.dram_tensor("wi", [128, cfg.WFREE], cfg.DT,
                             kind="ExternalInput").ap(),
        "out": nc.dram_tensor("out", [cfg.NTOK, cfg.RSH], cfg.DT,
                              kind="ExternalOutput").ap(),
    }
    if cfg.ND8:
        # fp8 chunk-pair tiles for chunks NK16..NK-1, same block layout
        aps["xT8"] = nc.dram_tensor(
            "xT8", [cfg.ND8 * cfg.NTG * 128, 2 * cfg.TOKG], F8E4,
            kind="ExternalInput").ap()
    with tile.TileContext(nc) as tc:
        with ExitStack() as ctx:
            build_body(ctx, tc, cfg, aps)
    nc.compile()
    return nc


def host_prep(cfg: Cfg, x, rows, cols, w_real, w_imag, phase_angles):
    """Host prep: transpose/tile x (fp16 pairs + fp8 pairs for the
    DoubleRow chunks), scatter-add the COO edges into dense Wr/Wi, and
    slice/layout per-core W.T tiles. Returns per-core input maps."""
    x = np.ascontiguousarray(np.asarray(x, dtype=np.float32)).reshape(
        cfg.NTOK, cfg.F)
    xT = x.T  # [F, NTOK] f32

    def pair_tiles(xpart, np_dt):
        npair = xpart.shape[0] // 256
        return np.ascontiguousarray(
            xpart.reshape(npair, 2, 128, cfg.NTG, cfg.TOKG)
            .transpose(0, 3, 2, 1, 4)
        ).reshape(npair * cfg.NTG * 128, 2 * cfg.TOKG).astype(np_dt)

    xT2 = pair_tiles(xT[:cfg.NK16 * 128], cfg.DT_NP)

    rows = np.asarray(rows).astype(np.int64, copy=False)
    cols = np.asarray(cols).astype(np.int64, copy=False)
    lin = rows * cfg.F + cols
    ncell = cfg.RTOT * cfg.F
    Wr = np.bincount(lin, weights=np.asarray(w_real, np.float64),
                     minlength=ncell).astype(np.float32).reshape(
        cfg.RTOT, cfg.F)
    Wi = np.bincount(lin, weights=np.asarray(w_imag, np.float64),
                     minlength=ncell).astype(np.float32).reshape(
        cfg.RTOT, cfg.F)

    # per-core W.T layout: arr[c, p, k*RSH + r] = W[c*RSH + r, k*128 + p]
    def wt_layout(W):
        return np.ascontiguousarray(
            W.T.reshape(cfg.NK, 128, cfg.NCORES, cfg.RSH)
            .transpose(2, 1, 0, 3)
        ).reshape(cfg.NCORES, 128, cfg.WFREE).astype(cfg.DT_NP)

    wr_t = wt_layout(Wr)
    wi_t = wt_layout(Wi)

    # pre-folded Sin argument: q = phase + pi/2 wrapped into [-pi, pi],
    # broadcast to all 128 partitions (device computes cos = Sin(q))
    q = np.asarray(phase_angles, dtype=np.float32) + np.float32(math.pi / 2)
    q = np.where(q > math.pi, q - np.float32(2 * math.pi), q)
    phase_in = np.ascontiguousarray(
        np.broadcast_to(q.astype(np.float32), (128, cfg.NBATCH)))

    in_maps = []
    for cid in range(cfg.NCORES):
        m = {"xT2": xT2, "phase": phase_in,
             "wr": wr_t[cid], "wi": wi_t[cid]}
        in_maps.append(m)
    if cfg.ND8:
        xT8 = pair_tiles(xT[cfg.NK16 * 128:], ml_dtypes.float8_e4m3fn)
        for m in in_maps:
            m["xT8"] = xT8
    return in_maps


_NC_CACHE = {}
LAST_RESULTS = None  # BassKernelResults of the most recent kernel() call


def kernel(x, rows, cols, w_real, w_imag, phase_angles, out_features=4096,
           **_ignored):
    from concourse.bass_utils import run_bass_kernel_spmd

    global LAST_RESULTS
    cfg = Cfg()
    assert int(out_features) == cfg.RTOT

    if "nc" not in _NC_CACHE:
        _NC_CACHE["nc"] = build_nc(cfg)
    nc = _NC_CACHE["nc"]

    in_maps = host_prep(cfg, x, rows, cols, w_real, w_imag, phase_angles)
    res = run_bass_kernel_spmd(nc, in_maps, core_ids=list(range(cfg.NCORES)))
    LAST_RESULTS = res
    out = np.concatenate(
        [res.results[c]["out"].astype(np.float32)
         for c in range(cfg.NCORES)], axis=1)
    return out.reshape(cfg.NTOK // 2048, 2048, cfg.RTOT)


# revision 34
# speedup vs baseline: 1.0231x; 1.0115x over previous
"""Trainium2 Bass kernel for nn_ComplexHoloLinear.

Computes out = x @ Wr.T + cos(phase)[batch] * (x @ Wi.T) where Wr/Wi are
dense [4096, 4096] matrices assembled from COO duplicates (host-side
scatter-add, per the sharding hint's "replicate the assembled sparse
weight"), distributed by output-feature sharding: each of the 8 cores owns
512 output rows.

Device pipeline (per core), structured so the PE never starves:
  - cos(phase) on device (DVE range-fold + ACT Sin LUT; a dummy Sin on a
    memset tile preloads the ACT table off the critical path), then
    incremental combine deltas dlt[b] = cos_b - cos_{b-1}.
  - Weights live in SBUF as [128, 32*512] fp16 (WB = combined, WI = imag).
    All DMAs move chunk PAIRS (256-512 KiB) to halve dispatch (~600ns per
    dma_start of sequencer time) and the end-of-kernel per-DMA semaphore
    drain. During the first sweep the three input streams ride three
    rings: wr->sync, wi->gpsimd, x->scalar.
  - Batch 0 runs token-groups 0+1 jointly in one k-sweep (8 PSUM banks),
    so chunk consumption (~1.7us/chunk) stays below first-load arrival
    (~1.1us/chunk) and the PE never waits on assembly.
  - Per-batch combine WB += dlt_b * WI rides chunk-wise behind the
    previous batch's last sweep (WAR deps); no double buffer, no stall.
  - The last NS8=8 feature chunks run as fp8-e4m3 DoubleRow matmuls
    (2 fp8 MACs/cell/cycle): x pairs come pre-cast from the host, the
    combined weight pairs are cast fp16->fp8 on DVE each batch. With 8 of
    32 chunks in fp8 the output rel err is ~1.9e-2 (budget 2e-2) and the
    PE stream drops ~50us.
  - PSUM -> SBUF staging casts to fp16 on DVE into one [128, 2048] tile
    per token group -> single gpsimd out DMA; host upcasts to f32.
"""

import math
from contextlib import ExitStack

import numpy as np
import ml_dtypes

import concourse.bass as bass
import concourse.tile as tile
from concourse import bacc, mybir

F32 = mybir.dt.float32
F16 = mybir.dt.float16
F8E4 = mybir.dt.float8e4
F8E3 = mybir.dt.float8e3
ADD = mybir.AluOpType.add
MULT = mybir.AluOpType.mult


class Cfg:
    """Full-size problem config."""

    NCORES = 8
    NTOK = 8192       # B * S tokens
    NBATCH = 4        # batches (distinct cos factors)
    F = 4096          # in features (contraction)
    RTOT = 4096       # out features
    TOKG = 512        # tokens per matmul sweep group (psum tiles of 128)
    NS8 = 8           # trailing feature chunks computed in fp8 DoubleRow

    @property
    def RSH(self):    # rows per core
        return self.RTOT // self.NCORES

    @property
    def NK(self):     # feature chunks of 128
        return self.F // 128

    @property
    def NK16(self):   # fp16 chunks
        return self.NK - self.NS8

    @property
    def NP16(self):   # fp16 chunk pairs
        return self.NK16 // 2

    @property
    def ND8(self):    # fp8 chunk pairs (DoubleRow double-chunks)
        return self.NS8 // 2

    @property
    def NTG(self):    # token groups
        return self.NTOK // self.TOKG

    @property
    def WFREE(self):  # W tile free size
        return self.NK * self.RSH

    @property
    def DT_NP(self):
        return np.float16

    @property
    def DT(self):
        return F16


def build_body(ctx: ExitStack, tc: tile.TileContext, cfg: Cfg, aps: dict):
    nc = tc.nc
    xT2 = aps["xT2"]        # [NP16*NTG*128, 2*TOKG] fp16 chunk-pair tiles
    xT2e = aps["xT2e"]      # [NP16*2*128, 2*TOKG] e3m4 pairs, tokens 0..1023
    phase = aps["phase"]    # [1, NBATCH]
    wr, wi = aps["wr"], aps["wi"]  # [128, WFREE] fp16
    out = aps["out"]        # [NTOK, RSH] fp16
    xT8 = aps.get("xT8")    # [ND8*NTG*128, 2*TOKG] fp8 chunk-pair tiles

    RSH, NK, NB = cfg.RSH, cfg.NK, cfg.NBATCH
    TPG = cfg.TOKG // 128   # psum tiles per token group
    DT = cfg.DT
    NPAIR = NK // 2

    wpool = ctx.enter_context(tc.tile_pool(name="w", bufs=1))
    x3pool = ctx.enter_context(tc.tile_pool(name="x3", bufs=16))
    xqpool = ctx.enter_context(tc.tile_pool(name="xq", bufs=5))
    tpool = ctx.enter_context(tc.tile_pool(name="tmp", bufs=2))
    spool = ctx.enter_context(tc.tile_pool(name="stage", bufs=3))
    mpool = ctx.enter_context(tc.tile_pool(name="misc", bufs=1))
    pspool = ctx.enter_context(tc.tile_pool(name="ps", bufs=2, space="PSUM"))
    if cfg.ND8:
        x8pool = ctx.enter_context(tc.tile_pool(name="x8", bufs=8))
        x8qpool = ctx.enter_context(tc.tile_pool(name="x8q", bufs=3))

    # Dummy Sin on a memset tile: preloads the ACT Sin LUT while the phase
    # DMA is still in flight (the table load is ~1.3us and otherwise lands
    # on the cos critical path).
    dum = mpool.tile([128, 1], F32)
    nc.vector.memset(dum[:], 0.0)
    dums = mpool.tile([128, 1], F32)
    nc.scalar.activation(dums[:], dum[:], mybir.ActivationFunctionType.Sin)

    # --- cos(phase) = Sin(q), where q = fold(phase + pi/2) into [-pi, pi]
    # comes pre-folded and pre-broadcast from the host (pure relayout of 4
    # scalars; the transcendental stays on device).
    ph = mpool.tile([128, NB], F32)
    nc.sync.dma_start(out=ph[:], in_=phase[:, :])
    cos_t = mpool.tile([128, NB], F32)
    nc.scalar.activation(cos_t[:], ph[:], mybir.ActivationFunctionType.Sin)

    # incremental deltas for b>=1 (b=0 uses cos_t[:, 0:1] directly)
    dlt = mpool.tile([128, NB], F32)
    if NB > 1:
        nc.vector.tensor_tensor(out=dlt[:, 1:NB], in0=cos_t[:, 1:NB],
                                in1=cos_t[:, 0:NB - 1],
                                op=mybir.AluOpType.subtract)

    WB = wpool.tile([128, cfg.WFREE], DT)   # combined weight (starts as Wr)
    WI = wpool.tile([128, cfg.WFREE], DT)
    w8s = [wpool.tile([128, 2 * RSH], F8E4, name=f"w8_{kd}")
           for kd in range(cfg.ND8)]

    ntg_per_b = cfg.NTG // NB

    def mm_sweep(b, tgs):
        """One k-sweep over all chunk pairs for token groups `tgs` of
        batch b. tgs[0]==0 sweeps also run the per-batch weight combine;
        the b==0,tg==0 sweep additionally DMAs the weights in."""
        sweep0 = b == 0 and tgs[0] == 0
        combine = tgs[0] == 0
        gts = [b * ntg_per_b + tg for tg in tgs]
        pts = {}
        for i in range(len(tgs)):
            pts[i] = [pspool.tile([128, RSH], F32, space="PSUM",
                                  tag=f"ps{t}", name=f"ps{i}_{t}")
                      for t in range(TPG)]
        hw = [nc.scalar, nc.sync]

        def w_load_combine(k2):
            """Weight DMA (sweep0) + per-batch combine for chunk-pair k2.
            Pair 0 is graded into single chunks so the first matmul's
            weights arrive (and combine) as early as possible."""
            segs = ([slice(0, RSH), slice(RSH, 2 * RSH)] if k2 == 0 else
                    [slice(k2 * 2 * RSH, (k2 + 1) * 2 * RSH)])
            for si, sg in enumerate(segs):
                if sweep0:
                    # dedicated rings: wr=sync, wi=gpsimd, x=scalar — the
                    # joint sweep runs at ~85% of HBM bandwidth and mixing
                    # streams on one FIFO ring delays the weight pairs
                    nc.sync.dma_start(out=WB[:, sg], in_=wr[:, sg])
                    nc.gpsimd.dma_start(out=WI[:, sg], in_=wi[:, sg])
                if combine:
                    csc = cos_t[:, 0:1] if b == 0 else dlt[:, b:b + 1]
                    n = sg.stop - sg.start
                    tmp = tpool.tile([128, 2 * RSH], DT, name="tmp")
                    nc.vector.tensor_scalar(out=tmp[:, :n], in0=WI[:, sg],
                                            scalar1=csc, scalar2=None,
                                            op0=MULT)
                    nc.vector.tensor_tensor(out=WB[:, sg], in0=WB[:, sg],
                                            in1=tmp[:, :n], op=ADD)
            if combine and k2 >= cfg.NP16:
                pr = slice(k2 * 2 * RSH, (k2 + 1) * 2 * RSH)
                nc.vector.tensor_copy(w8s[k2 - cfg.NP16][:], WB[:, pr])

        def fp16_mms(i, xap, xoff, k2, first, last):
            for j in range(2):
                sl = slice((2 * k2 + j) * RSH, (2 * k2 + j + 1) * RSH)
                for t in range(TPG):
                    nc.tensor.matmul(
                        out=pts[i][t][:],
                        lhsT=xap[:, xoff + j * cfg.TOKG + t * 128:
                                 xoff + j * cfg.TOKG + (t + 1) * 128],
                        rhs=WB[:, sl],
                        start=(first and j == 0),
                        stop=(last and j == 1),
                    )

        def fp8_mms(i, xap, xoff, kd, last):
            w3 = w8s[kd][:].rearrange("p (j r) -> p j r", j=2)
            x3 = xap[:, xoff:xoff + 2 * cfg.TOKG].rearrange(
                "p (j w) -> p j w", j=2)
            for t in range(TPG):
                nc.tensor.matmul(
                    out=pts[i][t][:],
                    lhsT=x3[:, :, t * 128:(t + 1) * 128],
                    rhs=w3,
                    start=False, stop=last,
                    perf_mode=mybir.MatmulPerfMode.DoubleRow,
                )

        x4d = xT2.rearrange("(k g p) c -> k g p c", g=cfg.NTG, p=128)
        x3_4d = xT2e.rearrange("(k g p) c -> k g p c", g=2, p=128)
        x8_4d = xT8.rearrange("(k g p) c -> k g p c", g=cfg.NTG, p=128) \
            if cfg.ND8 else None

        if len(gts) > 1 or sweep0:
            # pair-granular loads (three input streams on dedicated rings
            # while weights stream in)
            for k2 in range(NPAIR):
                fp8 = k2 >= cfg.NP16
                w_load_combine(k2)
                for i, gt in enumerate(gts):
                    xeng = nc.scalar if sweep0 else hw[(k2 + i) % 2]
                    if fp8:
                        kd = k2 - cfg.NP16
                        xt8 = x8pool.tile([128, 2 * cfg.TOKG], F8E4)
                        xeng.dma_start(out=xt8[:], in_=x8_4d[kd, gt, :, :])
                        fp8_mms(i, xt8[:], 0, kd, k2 == NPAIR - 1)
                    else:
                        # joint-sweep fp16 chunks read x as e3m4 (4-bit
                        # mantissa, PE runs it at fp16 rate): halves the x
                        # bytes in the only bandwidth-critical sweep.
                        # Error cost is global-norm tiny (2 of 16 token
                        # groups): 0.01874 -> 0.01906 measured.
                        xt = x3pool.tile([128, 2 * cfg.TOKG], F8E3)
                        xeng.dma_start(out=xt[:], in_=x3_4d[k2, gt, :, :])
                        fp16_mms(i, xt[:], 0, k2, k2 == 0, False)
        else:
            # quad-granular x loads (halves DMA dispatch + end-of-kernel
            # semaphore drain); one 4D-AP DMA covers two chunk-pair blocks
            gt = gts[0]
            for q in range(NPAIR // 2):
                k2a = 2 * q
                fp8 = k2a >= cfg.NP16
                if combine:
                    w_load_combine(k2a)
                    w_load_combine(k2a + 1)
                if fp8:
                    kd = k2a - cfg.NP16
                    xt8 = x8qpool.tile([128, 4 * cfg.TOKG], F8E4)
                    hw[q % 2].dma_start(
                        out=xt8[:],
                        in_=x8_4d[kd:kd + 2, gt, :, :].rearrange(
                            "u p c -> p u c"))
                    fp8_mms(0, xt8[:], 0, kd, False)
                    fp8_mms(0, xt8[:], 2 * cfg.TOKG, kd + 1,
                            k2a + 1 == NPAIR - 1)
                else:
                    xt = xqpool.tile([128, 4 * cfg.TOKG], DT)
                    hw[q % 2].dma_start(
                        out=xt[:],
                        in_=x4d[k2a:k2a + 2, gt, :, :].rearrange(
                            "u p c -> p u c"))
                    fp16_mms(0, xt[:], 0, k2a, k2a == 0, False)
                    fp16_mms(0, xt[:], 2 * cfg.TOKG, k2a + 1, False, False)
        for i, gt in enumerate(gts):
            stg = spool.tile([128, TPG * RSH], DT)
            tok0 = gt * cfg.TOKG
            last_sweep = b == NB - 1 and gt == cfg.NTG - 1
            for t in range(TPG):
                if last_sweep:
                    # final group: split casts over DVE+ACT so they run in
                    # parallel, and send per-tile out DMAs as each lands —
                    # both shorten the end-of-kernel drain
                    if t % 2 == 0:
                        nc.vector.tensor_copy(stg[:, t * RSH:(t + 1) * RSH],
                                              pts[i][t][:])
                    else:
                        nc.scalar.copy(out=stg[:, t * RSH:(t + 1) * RSH],
                                       in_=pts[i][t][:])
                    tk = tok0 + t * 128
                    hw[t % 2].dma_start(
                        out=out[tk:tk + 128, :],
                        in_=stg[:, t * RSH:(t + 1) * RSH])
                else:
                    nc.vector.tensor_copy(stg[:, t * RSH:(t + 1) * RSH],
                                          pts[i][t][:])
            if not last_sweep:
                # outs ride the HWDGE rings (idle mid-stream; faster
                # completion receipt than SWDGE and keeps the gpsimd
                # end-of-kernel semaphore ladder off the critical tail)
                dview = out[tok0:tok0 + cfg.TOKG, :].rearrange(
                    "(t p) r -> p t r", p=128)
                hw[gt % 2].dma_start(
                    out=dview, in_=stg[:].rearrange("p (t r) -> p t r",
                                                    t=TPG))

    for b in range(NB):
        sweeps = [[0, 1], [2], [3]] if b == 0 else [[0], [1], [2], [3]]
        for tgs in sweeps:
            mm_sweep(b, tgs)


def build_nc(cfg: Cfg):
    nc = bacc.Bacc("TRN2", target_bir_lowering=False, debug=False,
                   num_devices=cfg.NCORES)
    aps = {
        # x chunk-pair tiles: row block (k2*NTG + gt)*128 holds
        # [128 part, j*TOKG + t] = x[(2*k2+j)*128 + p, gt*TOKG + t]
        "xT2": nc.dram_tensor(
            "xT2", [cfg.NP16 * cfg.NTG * 128, 2 * cfg.TOKG], cfg.DT,
            kind="ExternalInput").ap(),
        # e3m4 copy of the fp16-chunk x pairs for token groups 0-1 only
        # (the bandwidth-critical joint first sweep)
        "xT2e": nc.dram_tensor(
            "xT2e", [cfg.NP16 * 2 * 128, 2 * cfg.TOKG], F8E3,
            kind="ExternalInput").ap(),
        "phase": nc.dram_tensor("phase", [128, cfg.NBATCH], F32,
                                kind="ExternalInput").ap(),
        "wr": nc.dram_tensor("wr", [128, cfg.WFREE], cfg.DT,
                             kind="ExternalInput").ap(),
        "wi": nc.dram_tensor("wi", [128, cfg.WFREE], cfg.DT,
                             kind="ExternalInput").ap(),
        "out": nc.dram_tensor("out", [cfg.NTOK, cfg.RSH], cfg.DT,
                              kind="ExternalOutput").ap(),
    }
    if cfg.ND8:
        # fp8 chunk-pair tiles for chunks NK16..NK-1, same block layout
        aps["xT8"] = nc.dram_tensor(
            "xT8", [cfg.ND8 * cfg.NTG * 128, 2 * cfg.TOKG], F8E4,
            kind="ExternalInput").ap()
    with tile.TileContext(nc) as tc:
        with ExitStack() as ctx:
            build_body(ctx, tc, cfg, aps)
    nc.compile()
    return nc


def host_prep(cfg: Cfg, x, rows, cols, w_real, w_imag, phase_angles):
    """Host prep: transpose/tile x (fp16 pairs + fp8 pairs for the
    DoubleRow chunks), scatter-add the COO edges into dense Wr/Wi, and
    slice/layout per-core W.T tiles. Returns per-core input maps."""
    x = np.ascontiguousarray(np.asarray(x, dtype=np.float32)).reshape(
        cfg.NTOK, cfg.F)
    xT = x.T  # [F, NTOK] f32

    def pair_tiles(xpart, np_dt, ntg=None):
        ntg = ntg or cfg.NTG
        npair = xpart.shape[0] // 256
        return np.ascontiguousarray(
            xpart.reshape(npair, 2, 128, ntg, cfg.TOKG)
            .transpose(0, 3, 2, 1, 4)
        ).reshape(npair * ntg * 128, 2 * cfg.TOKG).astype(np_dt)

    xT2 = pair_tiles(xT[:cfg.NK16 * 128], cfg.DT_NP)
    xT2e = pair_tiles(xT[:cfg.NK16 * 128, :2 * cfg.TOKG],
                      ml_dtypes.float8_e3m4, ntg=2)

    rows = np.asarray(rows).astype(np.int64, copy=False)
    cols = np.asarray(cols).astype(np.int64, copy=False)
    lin = rows * cfg.F + cols
    ncell = cfg.RTOT * cfg.F
    Wr = np.bincount(lin, weights=np.asarray(w_real, np.float64),
                     minlength=ncell).astype(np.float32).reshape(
        cfg.RTOT, cfg.F)
    Wi = np.bincount(lin, weights=np.asarray(w_imag, np.float64),
                     minlength=ncell).astype(np.float32).reshape(
        cfg.RTOT, cfg.F)

    # per-core W.T layout: arr[c, p, k*RSH + r] = W[c*RSH + r, k*128 + p]
    def wt_layout(W):
        return np.ascontiguousarray(
            W.T.reshape(cfg.NK, 128, cfg.NCORES, cfg.RSH)
            .transpose(2, 1, 0, 3)
        ).reshape(cfg.NCORES, 128, cfg.WFREE).astype(cfg.DT_NP)

    wr_t = wt_layout(Wr)
    wi_t = wt_layout(Wi)

    # pre-folded Sin argument: q = phase + pi/2 wrapped into [-pi, pi],
    # broadcast to all 128 partitions (device computes cos = Sin(q))
    q = np.asarray(phase_angles, dtype=np.float32) + np.float32(math.pi / 2)
    q = np.where(q > math.pi, q - np.float32(2 * math.pi), q)
    phase_in = np.ascontiguousarray(
        np.broadcast_to(q.astype(np.float32), (128, cfg.NBATCH)))

    in_maps = []
    for cid in range(cfg.NCORES):
        m = {"xT2": xT2, "xT2e": xT2e, "phase": phase_in,
             "wr": wr_t[cid], "wi": wi_t[cid]}
        in_maps.append(m)
    if cfg.ND8:
        xT8 = pair_tiles(xT[cfg.NK16 * 128:], ml_dtypes.float8_e4m3fn)
        for m in in_maps:
            m["xT8"] = xT8
    return in_maps


_NC_CACHE = {}
LAST_RESULTS = None  # BassKernelResults of the most recent kernel() call


def kernel(x, rows, cols, w_real, w_imag, phase_angles, out_features=4096,
           **_ignored):
    from concourse.bass_utils import run_bass_kernel_spmd

    global LAST_RESULTS
    cfg = Cfg()
    assert int(out_features) == cfg.RTOT

    if "nc" not in _NC_CACHE:
        _NC_CACHE["nc"] = build_nc(cfg)
    nc = _NC_CACHE["nc"]

    in_maps = host_prep(cfg, x, rows, cols, w_real, w_imag, phase_angles)
    res = run_bass_kernel_spmd(nc, in_maps, core_ids=list(range(cfg.NCORES)))
    LAST_RESULTS = res
    out = np.concatenate(
        [res.results[c]["out"].astype(np.float32)
         for c in range(cfg.NCORES)], axis=1)
    return out.reshape(cfg.NTOK // 2048, 2048, cfg.RTOT)
